# revision 1
# baseline (speedup 1.0000x reference)
"""MoE (top-2 of 8 experts, d=1024, h=4096) on 8 Trainium2 NeuronCores.

Strategy (expert-parallel, per sharding hint):
  - Host: gating (fp64 logits/softmax/top-2 — tie margins on this problem are
    ~1e-5, far above fp32 rounding noise, so host routing matches the
    reference's fp32 top-k), per-expert token gather, pad to capacity C.
  - Device (core e = expert e): hidT = relu(W1_e.T @ x_e.T + b1_e) then
    ye = hidT.T @ W2_e, both as K-tiled 128x128x512 matmuls in float32r
    (full PE rate, ~1e-4 matmul rel err).
  - Host: out[tok_e] += gate_e * (ye + b2_e)  (scatter-combine).

Self-contained: hardcodes all shapes; only imports concourse (system lib).
"""

import os

os.environ.setdefault("JAX_PLATFORMS", "")

import numpy as np

import concourse.bacc as bacc
import concourse.mybir as mybir
import concourse.tile as tile
from concourse.bass_utils import run_bass_kernel_spmd

P = 128
D = 1024  # embed dim
H = 4096  # hidden dim
E = 8  # experts
TOPK = 2
KD = D // P  # 8  k-tiles over embed
KH = H // P  # 32 k-tiles over hidden
NCORES = 8
FD = 512  # matmul moving free dim (one PSUM bank of fp32)

_compiled = {}
LAST_RESULT = None  # BassKernelResults of the most recent run (for test harness)


def _phase1(nc, tc, rs, C, chunks, xt_d, w1_d, b1_d, hid_cs):
    f32 = mybir.dt.float32
    f32r = mybir.dt.float32r
    relu = mybir.ActivationFunctionType.Relu
    TN = len(chunks)
    with (
        tc.tile_pool(name=rs + "xts_p", bufs=1) as xts_p,
        tc.tile_pool(name=rs + "b1_p", bufs=1) as b1_p,
        tc.tile_pool(name=rs + "w1_p", bufs=int(os.environ.get("MOE_W1B", "4"))) as w1_p,
        tc.tile_pool(name=rs + "hb_p", bufs=1) as hb_p,
        tc.tile_pool(name=rs + "ps1", bufs=int(os.environ.get("MOE_PS1", "4")), space="PSUM") as ps1,
    ):
        def load_w1(hm):
            w1t = w1_p.tile([P, KD, P], f32r, tag="w1t", name=rs + f"w1t_{hm}")
            nc.sync.dma_start(w1t[:], w1_d[:, hm])
            return w1t

        # Issue order matters: HWDGE dispatches in program order, so the
        # first matmul group's deps (w1t[0] + x chunk 0) are issued first.
        w1_pre = {0: load_w1(0)}
        # x chunks as separate per-k tiles so the first matmul group only
        # waits on its own 8 pieces (~2MB), not the whole 10MB load.
        xc = [[None] * KD for _ in range(TN)]
        for tn, (off, w) in enumerate(chunks):
            for k0 in range(0, KD, 2):
                t = xts_p.tile(
                    [P, 2, w], f32r, tag=f"x_{tn}_{k0}", name=rs + f"x_{tn}_{k0}"
                )
                nc.sync.dma_start(t[:], xt_d[:, k0 : k0 + 2, off : off + w])
                xc[tn][k0] = t[:, 0, :]
                xc[tn][k0 + 1] = t[:, 1, :]
            if tn == 0:
                # b1 is first needed at the first eviction, not the first
                # matmul: issue it after chunk 0's loads.
                b1s = b1_p.tile([P, KH], f32, name=rs + "b1s")
                nc.sync.dma_start(b1s[:], b1_d[:])
            if tn < 3:  # prefetch next stationary tiles early
                w1_pre[tn + 1] = load_w1(tn + 1)
        # PE emission order: the first W hm rows are swept tn-major (wave
        # order) so the earliest matmuls only touch x chunks that have
        # already landed; the rest are hm-major. Each (hm, tn) psum group is
        # independent, so this only reorders work.
        W = int(os.environ.get("MOE_W", "2")) if TN > 1 else 0
        sched = [(hm, tn) for tn in range(TN) for hm in range(W)]
        sched += [(hm, tn) for hm in range(W, KH) for tn in range(TN)]

        w1ts, done = {}, {}
        KQ1 = KH // 4
        for hm, tn in sched:
            if hm not in w1ts:
                w1ts[hm] = w1_pre.pop(hm) if hm in w1_pre else load_w1(hm)
                done[hm] = 0
            off, w = chunks[tn]
            pt = ps1.tile([P, FD], f32, tag="ps1", name=rs + f"ps1_{hm}_{tn}")
            for k in range(KD):
                nc.tensor.matmul(
                    pt[:, :w],
                    w1ts[hm][:, k, :],
                    xc[tn][k],
                    start=(k == 0),
                    stop=(k == KD - 1),
                )
            # evict through a small per-chunk staging tile (ACT does
            # relu+bias, then the hid write DMAs it straight out on the ACT
            # HWDGE ring so phase-2 loads (SP ring) aren't queued behind it)
            hbst = int(os.environ.get("MOE_HBST", "12")) if C <= 2560 else 6
            hb = hb_p.tile([P, w], f32r, tag="hbst", bufs=hbst, name=rs + f"hb_{hm}_{tn}")
            nc.scalar.activation(
                hb[:, :w], pt[:, :w], relu, bias=b1s[:, hm : hm + 1]
            )
            nc.scalar.dma_start(
                hid_cs[tn][hm // KQ1][:, :, hm % KQ1, :].transpose([1, 0, 2]),
                hb.rearrange("p (t q) -> p t q", q=P),
            )
            done[hm] += 1
            if done[hm] == TN:
                del w1ts[hm]  # release references; pool slots recycle


W2HEAD = 8  # w2 chunks living in the persistent pool (loadable during phase 1)


def _phase2(nc, tc, rs, C, chunks, w2_d, hid_cs, ye_d, hd_p, w2h_p, ps2):
    f32 = mybir.dt.float32
    f32r = mybir.dt.float32r
    TM = C // P
    with (
        tc.tile_pool(name=rs + "w2_p", bufs=1) as w2_p,
        tc.tile_pool(name=rs + "out_p", bufs=int(os.environ.get("MOE_OUTB", "3"))) as out_p,
    ):

        HDS = 4  # hd k-split (must match the 4-way hid_cs DRAM split)
        KQ = KH // HDS

        def load_hd(tm):
            cidx = next(
                i for i, (off, w) in enumerate(chunks) if off // P <= tm < (off + w) // P
            )
            local = tm - chunks[cidx][0] // P
            parts = []
            for q in range(HDS):
                hdq = hd_p.tile(
                    [P, KQ, P], f32r, tag=f"hd{q}", name=rs + f"hd_{tm}_{q}"
                )
                nc.sync.dma_start(hdq[:], hid_cs[cidx][q][local])
                parts.append(hdq)
            return parts

        # Issue order: w2 head + first token tile's data before the bulk w2
        # load, so the first phase-2 matmul isn't queued behind 16MB of w2 on
        # the in-order HWDGE ring. Head w2 + hd live in pools hoisted outside
        # phase 1's, so these loads can run during phase 1's tail.
        w2ts = []
        for k in range(W2HEAD):
            w2t = w2h_p.tile([P, D], f32r, tag=f"w2_{k}", name=rs + f"w2_{k}")
            nc.sync.dma_start(w2t[:], w2_d[k])
            w2ts.append(w2t)
        hd_pre = {0: load_hd(0)}
        for k in range(W2HEAD, KH):
            w2t = w2_p.tile([P, D], f32r, tag=f"w2_{k}", name=rs + f"w2_{k}")
            nc.sync.dma_start(w2t[:], w2_d[k])
            w2ts.append(w2t)
            if k == 15:
                hd_pre[1] = load_hd(1)
        hd_pre[2] = load_hd(2)
        for tm in range(TM):
            hd = hd_pre.pop(tm) if tm in hd_pre else load_hd(tm)
            ob = out_p.tile([P, D], f32, tag="ob", name=rs + f"ob_{tm}")
            for n in range(D // FD):
                pt2 = ps2.tile([P, FD], f32, tag="ps2", name=rs + f"ps2_{tm}_{n}")
                for k in range(KH):
                    nc.tensor.matmul(
                        pt2[:],
                        hd[k // KQ][:, k % KQ, :],
                        w2ts[k][:, n * FD : (n + 1) * FD],
                        start=(k == 0),
                        stop=(k == KH - 1),
                    )
                nc.vector.tensor_copy(ob[:, n * FD : (n + 1) * FD], pt2[:])
            nc.scalar.dma_start(ye_d[tm], ob[:])


def _build(C, reps=1):
    """Per-core SPMD program for capacity-C tokens through one expert.

    reps>1 repeats the whole program back-to-back (timing experiments only).
    """
    if (C, reps) in _compiled:
        return _compiled[(C, reps)]
    f32 = mybir.dt.float32
    f32r = mybir.dt.float32r
    TM = C // P  # token tiles (GEMM2 stationary / output rows)
    # GEMM1 moving chunks: 512s plus one remainder (multiple of 128; N>=256
    # keeps fp32r at full rate, a 128 tail is negligible)
    chunks = []
    off = 0
    CW = int(os.environ.get("MOE_CW", "0"))
    if CW and C % CW == 0:  # uniform chunk-width experiment knob
        while off < C:
            chunks.append((off, CW))
            off += CW
    else:
        if C >= 768:  # small first chunk -> first matmul group starts sooner
            chunks.append((0, 256))
            off = 256
        while off < C:
            w = min(FD, C - off)
            chunks.append((off, w))
            off += w

    nc = bacc.Bacc(None, target_bir_lowering=False)
    # xt host layout [P, KD, C]: xt[p, k, c] = x[tok_c, k*128+p] (transposed)
    xt_d = nc.dram_tensor("xt", [P, KD, C], f32r, kind="ExternalInput")
    # w1 host layout [P, KH, KD, P]: w1[p, hm, k, j] = W1[k*128+p, hm*128+j]
    # -> per-hm stationary-tile loads are contiguous 4KB per partition.
    w1_d = nc.dram_tensor("w1", [P, KH, KD, P], f32r, kind="ExternalInput")
    b1_d = nc.dram_tensor("b1", [P, KH], f32, kind="ExternalInput")
    w2_d = nc.dram_tensor("w2", [KH, P, D], f32r, kind="ExternalInput")
    ye_d = nc.dram_tensor("ye", [TM, P, D], f32, kind="ExternalOutput")

    with tile.TileContext(nc) as tc:
        with tc.tile_pool(name="dram", bufs=1, space="DRAM") as dram:
            # hidT blocks: [token-tile, hidden-in-tile (partition), hm,
            # token-in-tile] -> phase-2 reads are contiguous 16KB/partition.
            # One DRAM tile per token chunk so phase-2's first loads only
            # depend on writes to their own chunk.
            # ... and per k-quarter, so phase-2's early hd quarters depend
            # only on the phase-1 rows that produced them (DRAM deps are
            # whole-tile).
            hid_cs = [
                [
                    dram.tile(
                        [w // P, P, KH // 4, P],
                        f32r,
                        tag=f"hidc_{i}_{q}",
                        name=f"hidc_{i}_{q}",
                    )
                    for q in range(4)
                ]
                for i, (off, w) in enumerate(chunks)
            ]
            for rep in range(reps):
                rs = "" if rep == 0 else f"r{rep}_"
                # hd/w2-head/psum2 pools are hoisted outside phase 1's pools
                # so phase 2's first loads don't wait for phase-1 SBUF release.
                with (
                    tc.tile_pool(name=rs + "hd_p", bufs=3) as hd_p,
                    tc.tile_pool(name=rs + "w2h_p", bufs=1) as w2h_p,
                    tc.tile_pool(name=rs + "ps2", bufs=int(os.environ.get("MOE_PS2", "4")), space="PSUM") as ps2,
                ):
                    _phase1(nc, tc, rs, C, chunks, xt_d, w1_d, b1_d, hid_cs)
                    _phase2(
                        nc, tc, rs, C, chunks, w2_d, hid_cs, ye_d, hd_p, w2h_p, ps2
                    )

    nc.compile()
    _compiled[(C, reps)] = nc
    return nc


def kernel(x, Wg, bg, W1, b1, W2, b2):
    global LAST_RESULT
    x = np.ascontiguousarray(x, dtype=np.float32)
    B, S, d = x.shape
    assert d == D
    T = B * S
    xf = x.reshape(T, d)

    # ---- Host gating/routing (fp64) ----
    logits = xf.astype(np.float64) @ Wg.astype(np.float64) + bg.astype(np.float64)
    mx = logits.max(axis=1, keepdims=True)
    ex = np.exp(logits - mx)
    probs = ex / ex.sum(axis=1, keepdims=True)
    order = np.argsort(-logits, axis=1, kind="stable")  # ties -> lower index
    top = order[:, :TOPK]  # [T, 2]
    gsel = np.take_along_axis(probs, top, axis=1).astype(np.float32)

    toks, gates = [], []
    for e in range(E):
        pos = top == e  # [T, 2]
        sel = pos.any(axis=1)
        toks.append(np.nonzero(sel)[0])
        gates.append((gsel * pos).sum(axis=1)[sel].astype(np.float32))

    maxcnt = max(len(t) for t in toks)
    # SBUF budget caps resident x at 4096 tokens/core; batch if routing is
    # ever concentrated enough to exceed that (never for balanced gating).
    MAXC = 2944
    nb = max(1, -(-maxcnt // MAXC))
    C = max(P, ((-(-maxcnt // nb) + P - 1) // P) * P)

    w_maps = []  # per-expert weight shards (batch-invariant)
    for e in range(E):
        w_maps.append(
            {
                "w1": np.ascontiguousarray(
                    np.asarray(W1[e], dtype=np.float32)
                    .reshape(KD, P, KH, P)
                    .transpose(1, 2, 0, 3)
                ),
                "b1": np.ascontiguousarray(
                    np.asarray(b1[e], dtype=np.float32).reshape(KH, P).T
                ),
                "w2": np.ascontiguousarray(W2[e], dtype=np.float32).reshape(KH, P, D),
            }
        )

    nc = _build(C)
    out = np.zeros((T, D), np.float32)
    b2f = np.asarray(b2, dtype=np.float32)
    for b in range(nb):
        in_maps = []
        btoks = []
        for e in range(E):
            tk = toks[e][b * C : (b + 1) * C]
            btoks.append(tk)
            xe = np.zeros((C, D), np.float32)
            xe[: len(tk)] = xf[tk]
            in_maps.append(
                {
                    "xt": np.ascontiguousarray(
                        xe.T.reshape(KD, P, C).transpose(1, 0, 2)
                    ),
                    **w_maps[e],
                }
            )
        res = run_bass_kernel_spmd(nc, in_maps, core_ids=list(range(NCORES)))
        LAST_RESULT = res
        for e in range(E):
            cnt = len(btoks[e])
            if cnt == 0:
                continue
            ye = res.results[e]["ye"].reshape(C, D)[:cnt]
            g = gates[e][b * C : b * C + cnt]
            out[btoks[e]] += g[:, None] * (ye + b2f[e])
    return out.reshape(B, S, D)



# revision 3
# speedup vs baseline: 1.4661x; 1.4661x over previous
"""MoE (top-2 of 8 experts, d=1024, h=4096) on 8 Trainium2 NeuronCores.

Strategy (hidden-dim sharding + fp8 DoubleRow split matmuls):
  - Host: gating in fp64 (tie margins ~1e-5 >> fp32 noise, so top-2 matches
    the reference), token-pair list ordered by expert (each expert's count
    padded to a multiple of 16), power-of-2 scaling + e4m3 hi/lo splitting
    of x and all weights.
  - Each core processes ALL 16384 token-expert pairs but only a 512-wide
    slice of the hidden dim (h-shard) of every expert -> perfect load
    balance (zero capacity padding), identical SPMD program on all cores.
  - GEMM1 (x @ W1_slice) and GEMM2 (hid_slice @ W2_slice) both run as fp8
    DoubleRow matmuls (two independent 128-deep contractions summed per
    instruction at 0.5 cycles/row). The 3-term split
        x @ W ~= Wh.T@(xh+xl) + Wl.T@xh
    costs 0.75x of one bf16 matmul at ~1e-3 accuracy:
      hi pass: lhsT slots (Wh, Wh) x rhs slots (xh, xl)   [1 DR / k-tile]
      lo pass: lhsT slots (Wl_2j, Wl_2j+1) x rhs (xh_2j, xh_2j+1)
                                                          [1 DR / 2 k-tiles]
  - hid stays in SBUF: ACT evicts psum1 -> t = relu(scale*psum+b1) bf16;
    gpsimd casts hh = fp8(t); DVE computes hl = fp8(t - hh). GEMM2 reads
    (hh, hl) slots. psum2 evicted to bf16 (DVE/ACT alternating) and DMAd
    out as partial sums over the h-shard; host sums the 8 partials and
    applies gates + b2.

Self-contained: hardcodes all shapes; only imports concourse (system lib).
"""

import os

os.environ.setdefault("JAX_PLATFORMS", "")

import numpy as np
import ml_dtypes

import concourse.bacc as bacc
import concourse.mybir as mybir
import concourse.tile as tile
from concourse.bass_utils import run_bass_kernel_spmd

F8 = ml_dtypes.float8_e4m3

P = 128
D = 1024  # embed dim
H = 4096  # hidden dim
E = 8  # experts
TOPK = 2
NCORES = 8
HS = H // NCORES  # 512: hidden slice per core
KD = D // P  # 8: k-tiles over embed (GEMM1 contraction)
KH = HS // P  # 4: h-tiles in the local slice (GEMM2 contraction)
DT = D // P  # 8: output d-tiles (GEMM2 output)
CW = 512  # chunk width (tokens per moving block; one PSUM bank of fp32)
SH = 32.0  # 2**5 fixed scale for hid in fp8

f32 = mybir.dt.float32
bf16 = mybir.dt.bfloat16
f8 = mybir.dt.float8e4
DR = mybir.MatmulPerfMode.DoubleRow
RELU = mybir.ActivationFunctionType.Relu
MULT = mybir.AluOpType.mult
SUB = mybir.AluOpType.subtract

_compiled = {}
LAST_RESULT = None  # BassKernelResults of the most recent run (for test harness)


def _g1(nc, ps1, chunk, xs, w1h, w1l, b1s, t_p, hs):
    """GEMM1 for one chunk + eviction/split of its hid slice."""
    (ci, e, off, w, s1) = chunk
    for m in range(KH // 2):  # hm pairs
        pt = ps1.tile([P, 2, CW], f32, tag="ps1", name=f"ps1_{ci}_{m}")
        for i in range(2):
            hm = 2 * m + i
            for k in range(KD):
                nc.tensor.matmul(
                    pt[:, i, :w],
                    w1h[:, hm, k],
                    xs[:, k, :, :w],
                    start=(k == 0),
                    stop=False,
                    perf_mode=DR,
                )
            for j in range(KD // 2):
                nc.tensor.matmul(
                    pt[:, i, :w],
                    w1l[:, hm, j],
                    xs[:, 2 * j : 2 * j + 2, 0, :w],
                    start=False,
                    stop=(j == KD // 2 - 1),
                    perf_mode=DR,
                )
        # t = relu(psum*s1 + b1) in bf16, then split into fp8 hi/lo slots
        t = t_p.tile([P, 2, CW], bf16, tag="t", name=f"t_{ci}_{m}")
        for i in range(2):
            hm = 2 * m + i
            nc.scalar.activation(
                t[:, i, :w], pt[:, i, :w], RELU,
                bias=b1s[:, KH * e + hm : KH * e + hm + 1],
                scale=s1,
            )
        nc.gpsimd.tensor_copy(hs[:, 2 * m : 2 * m + 2, 0, :w], t[:, :, :w])
        nc.vector.scalar_tensor_tensor(
            hs[:, 2 * m : 2 * m + 2, 1, :w], t[:, :, :w], 1.0,
            hs[:, 2 * m : 2 * m + 2, 0, :w],
            op0=MULT, op1=SUB,
        )


def _g2(nc, ps2, chunk, hs, w2h, w2l, ob):
    """GEMM2 for one chunk: 8 d-tiles of partial output."""
    (ci, e, off, w, s1) = chunk
    for dt in range(DT):
        pt = ps2.tile([P, CW], f32, tag="ps2", name=f"ps2_{ci}_{dt}")
        for k in range(KH):
            nc.tensor.matmul(
                pt[:, :w],
                w2h[:, dt, k],
                hs[:, k, :, :w],
                start=(k == 0),
                stop=False,
                perf_mode=DR,
            )
        for j in range(KH // 2):
            nc.tensor.matmul(
                pt[:, :w],
                w2l[:, dt, j],
                hs[:, 2 * j : 2 * j + 2, 0, :w],
                start=False,
                stop=(j == KH // 2 - 1),
                perf_mode=DR,
            )
        if dt % 2 == 0:
            nc.vector.tensor_copy(ob[:, dt, :w], pt[:, :w])
        else:
            nc.scalar.copy(ob[:, dt, :w], pt[:, :w])


def _build(chunks, npp):
    """Per-core SPMD program.

    chunks: list of (ci, expert, pair-offset, width, act_scale) covering
    [0, npp).
    """
    key = (npp, tuple(c[1:] for c in chunks))
    if key in _compiled:
        return _compiled[key]

    nc = bacc.Bacc(None, target_bir_lowering=False)
    xs_d = nc.dram_tensor("xs", [P, KD, 2, npp], f8, kind="ExternalInput")
    w1h_d = nc.dram_tensor("w1h", [E, P, KH, KD, 2, P], f8, kind="ExternalInput")
    w1l_d = nc.dram_tensor("w1l", [E, P, KH, KD // 2, 2, P], f8, kind="ExternalInput")
    w2h_d = nc.dram_tensor("w2h", [E, P, DT, KH, 2, P], f8, kind="ExternalInput")
    w2l_d = nc.dram_tensor("w2l", [E, P, DT, KH // 2, 2, P], f8, kind="ExternalInput")
    b1_d = nc.dram_tensor("b1", [P, E * KH], f32, kind="ExternalInput")
    out_d = nc.dram_tensor("out", [P, DT, npp], bf16, kind="ExternalOutput")

    n = len(chunks)

    with tile.TileContext(nc) as tc:
        with (
            tc.tile_pool(name="xs_p", bufs=3) as xs_p,
            tc.tile_pool(name="w_p", bufs=3) as w_p,
            tc.tile_pool(name="t_p", bufs=2) as t_p,
            tc.tile_pool(name="hs_p", bufs=3) as hs_p,
            tc.tile_pool(name="ob_p", bufs=3) as ob_p,
            tc.tile_pool(name="b1_p", bufs=1) as b1_p,
            tc.tile_pool(name="ps1", bufs=2, space="PSUM") as ps1,
            tc.tile_pool(name="ps2", bufs=4, space="PSUM") as ps2,
        ):

            def load_w(e):
                w1h = w_p.tile([P, KH, KD, 2, P], f8, tag="w1h", name=f"w1h_{e}")
                w1l = w_p.tile([P, KH, KD // 2, 2, P], f8, tag="w1l", name=f"w1l_{e}")
                w2h = w_p.tile([P, DT, KH, 2, P], f8, tag="w2h", name=f"w2h_{e}")
                w2l = w_p.tile([P, DT, KH // 2, 2, P], f8, tag="w2l", name=f"w2l_{e}")
                nc.sync.dma_start(w1h[:], w1h_d[e])
                nc.sync.dma_start(w1l[:], w1l_d[e])
                nc.sync.dma_start(w2h[:], w2h_d[e])
                nc.sync.dma_start(w2l[:], w2l_d[e])
                return (w1h, w1l, w2h, w2l)

            def load_xs(c):
                (ci, e, off, w, s1) = c
                xs = xs_p.tile([P, KD, 2, CW], f8, tag="xs", name=f"xs_{ci}")
                nc.sync.dma_start(xs[:, :, :, :w], xs_d[:, :, :, off : off + w])
                return xs

            b1s = b1_p.tile([P, E * KH], f32, name="b1s")
            nc.sync.dma_start(b1s[:], b1_d[:])
            wmap = {chunks[0][1]: load_w(chunks[0][1])}
            xmap = {0: load_xs(chunks[0])}
            for c in chunks[1:3]:
                if c[1] not in wmap:
                    wmap[c[1]] = load_w(c[1])
                xmap[c[0]] = load_xs(c)

            hsm = {}

            def emit_g1(c):
                (ci, e, off, w, s1) = c
                hs = hs_p.tile([P, KH, 2, CW], f8, tag="hs", name=f"hs_{ci}")
                hsm[ci] = hs
                _g1(nc, ps1, c, xmap.pop(ci), wmap[e][0], wmap[e][1], b1s, t_p, hs)

            def emit_g2(c):
                (ci, e, off, w, s1) = c
                ob = ob_p.tile([P, DT, CW], bf16, tag="ob", name=f"ob_{ci}")
                _g2(nc, ps2, c, hsm.pop(ci), wmap[e][2], wmap[e][3], ob)
                nc.scalar.dma_start(out_d[:, :, off : off + w], ob[:, :, :w])

            # software pipeline: G1(i+1) is emitted before G2(i) so the PE
            # never waits on the ACT/Pool/DVE hid-split chain.
            emit_g1(chunks[0])
            for i in range(n):
                if i + 2 < n:
                    c2 = chunks[i + 2]
                    xmap[c2[0]] = load_xs(c2)
                    if c2[1] not in wmap:
                        wmap[c2[1]] = load_w(c2[1])
                if i + 1 < n:
                    emit_g1(chunks[i + 1])
                emit_g2(chunks[i])
                ce = chunks[i][1]
                if i + 1 == n or chunks[i + 1][1] != ce:
                    del wmap[ce]

    nc.compile()
    _compiled[key] = nc
    return nc


def _quant_split(a):
    """e4m3 hi/lo split of a pre-scaled float32 array."""
    hi = a.astype(F8)
    lo = (a - hi.astype(np.float32)).astype(F8)
    return hi, lo


def _pow2_scale(maxval, target=160.0):
    return float(2.0 ** np.floor(np.log2(target / maxval)))


def kernel(x, Wg, bg, W1, b1, W2, b2):
    global LAST_RESULT
    x = np.ascontiguousarray(x, dtype=np.float32)
    B, S, d = x.shape
    assert d == D
    T = B * S
    xf = x.reshape(T, d)

    # ---- Host gating/routing (fp64) ----
    logits = xf.astype(np.float64) @ np.asarray(Wg, np.float64) + np.asarray(
        bg, np.float64
    )
    mx = logits.max(axis=1, keepdims=True)
    ex = np.exp(logits - mx)
    probs = ex / ex.sum(axis=1, keepdims=True)
    order = np.argsort(-logits, axis=1, kind="stable")  # ties -> lower index
    top = order[:, :TOPK]  # [T, 2]
    gsel = np.take_along_axis(probs, top, axis=1).astype(np.float32)

    toks, gates = [], []
    for e in range(E):
        pos = top == e  # [T, 2]
        sel = pos.any(axis=1)
        toks.append(np.nonzero(sel)[0])
        gates.append((gsel * pos).sum(axis=1)[sel].astype(np.float32))

    # pair layout: expert-major, each expert's count padded to a multiple
    # of 16 (DoubleRow AP alignment)
    cnt = [len(t) for t in toks]
    cnt16 = [-(-c // 16) * 16 for c in cnt]
    offs = np.concatenate([[0], np.cumsum(cnt16)]).astype(np.int64)
    npp = int(offs[-1])
    pair_tok = np.zeros(npp, np.int64)
    for e in range(E):
        pair_tok[offs[e] : offs[e] + cnt[e]] = toks[e]

    # ---- scales (powers of 2; lossless to apply) ----
    sx = _pow2_scale(np.abs(xf).max())
    sw1 = _pow2_scale(np.abs(W1).max())
    sw2 = _pow2_scale(np.abs(W2).max())
    s1 = SH / (sx * sw1)  # ACT scale: psum1 -> hid*SH
    inv_out = 1.0 / (SH * sw2)

    chunks = []
    ci = 0
    for e in range(E):
        off = int(offs[e])
        rem = cnt16[e]
        while rem > 0:
            w = min(CW, rem)
            chunks.append((ci, e, off, w, s1))
            ci += 1
            off += w
            rem -= w

    # ---- x: gather pairs, scale, split, arrange [P, KD, 2, npp] ----
    xg = xf[pair_tok] * sx
    xh, xl = _quant_split(xg)
    xs_host = np.empty((P, KD, 2, npp), F8)
    xs_host[:, :, 0, :] = xh.reshape(npp, KD, P).transpose(2, 1, 0)
    xs_host[:, :, 1, :] = xl.reshape(npp, KD, P).transpose(2, 1, 0)

    # ---- per-core weight shards ----
    W1f = np.asarray(W1, np.float32) * sw1
    W2f = np.asarray(W2, np.float32) * sw2
    b1f = np.asarray(b1, np.float32) * SH
    core_maps = []
    for c in range(NCORES):
        sl = slice(c * HS, (c + 1) * HS)
        w1hi, w1lo = _quant_split(W1f[:, :, sl])  # [E, D, HS]
        w2hi, w2lo = _quant_split(W2f[:, sl, :])  # [E, HS, D]
        # GEMM1 stationary: [e, p(d-in-k), hm, k, slot, j(h-in-hm)]
        a = w1hi.reshape(E, KD, P, KH, P).transpose(0, 2, 3, 1, 4)  # [E,p,hm,k,j]
        w1h_host = np.ascontiguousarray(
            np.broadcast_to(a[:, :, :, :, None, :], (E, P, KH, KD, 2, P))
        )
        bl = w1lo.reshape(E, KD, P, KH, P).transpose(0, 2, 3, 1, 4)
        w1l_host = np.ascontiguousarray(bl.reshape(E, P, KH, KD // 2, 2, P))
        # GEMM2 stationary: [e, p(h-in-k), dt, k, slot, j(d-in-dt)]
        a2 = w2hi.reshape(E, KH, P, DT, P).transpose(0, 2, 3, 1, 4)  # [E,p,dt,k,j]
        w2h_host = np.ascontiguousarray(
            np.broadcast_to(a2[:, :, :, :, None, :], (E, P, DT, KH, 2, P))
        )
        b2l = w2lo.reshape(E, KH, P, DT, P).transpose(0, 2, 3, 1, 4)
        w2l_host = np.ascontiguousarray(b2l.reshape(E, P, DT, KH // 2, 2, P))
        b1_host = np.ascontiguousarray(
            b1f[:, sl].reshape(E, KH, P).transpose(2, 0, 1).reshape(P, E * KH)
        )
        core_maps.append(
            {
                "xs": xs_host,
                "w1h": w1h_host,
                "w1l": w1l_host,
                "w2h": w2h_host,
                "w2l": w2l_host,
                "b1": b1_host,
            }
        )

    nc = _build(chunks, npp)
    res = run_bass_kernel_spmd(nc, core_maps, core_ids=list(range(NCORES)))
    LAST_RESULT = res

    # ---- combine partials on host ----
    total = np.zeros((P, DT, npp), np.float32)
    for c in range(NCORES):
        total += np.asarray(res.results[c]["out"]).astype(np.float32)
    # [p, dt, pair] -> [pair, dt*128=d]
    ytot = total.transpose(2, 1, 0).reshape(npp, D) * inv_out

    out = np.zeros((T, D), np.float32)
    b2f = np.asarray(b2, np.float32)
    for e in range(E):
        if cnt[e] == 0:
            continue
        ye = ytot[offs[e] : offs[e] + cnt[e]]
        out[toks[e]] += gates[e][:, None] * (ye + b2f[e])
    return out.reshape(B, S, D)


# revision 8
# speedup vs baseline: 1.5344x; 1.0465x over previous
"""MoE (top-2 of 8 experts, d=1024, h=4096) on 8 Trainium2 NeuronCores.

Strategy (hidden-dim sharding + fp8 DoubleRow split matmuls):
  - Host: gating in fp64 (tie margins ~1e-5 >> fp32 noise, so top-2 matches
    the reference), token-pair list ordered by expert (each expert's count
    padded to a multiple of 16), power-of-2 scaling + e4m3 hi/lo splitting
    of x and all weights.
  - Each core processes ALL 16384 token-expert pairs but only a 512-wide
    slice of the hidden dim (h-shard) of every expert -> perfect load
    balance (zero capacity padding), identical SPMD program on all cores.
  - GEMM1 (x @ W1_slice) and GEMM2 (hid_slice @ W2_slice) both run as fp8
    DoubleRow matmuls (two independent 128-deep contractions summed per
    instruction at 0.5 cycles/row). The 3-term split
        x @ W ~= Wh.T@(xh+xl) + Wl.T@xh
    costs 0.75x of one bf16 matmul at ~1e-3 accuracy:
      hi pass: lhsT slots (Wh, Wh) x rhs slots (xh, xl)   [1 DR / k-tile]
      lo pass: lhsT slots (Wl_2j, Wl_2j+1) x rhs (xh_2j, xh_2j+1)
                                                          [1 DR / 2 k-tiles]
  - hid stays in SBUF: ACT evicts psum1 -> t = relu(scale*psum+b1) bf16;
    gpsimd casts hh = fp8(t); DVE computes hl = fp8(t - hh). GEMM2 reads
    (hh, hl) slots. psum2 evicted to bf16 (DVE/ACT alternating) and DMAd
    out as partial sums over the h-shard; host sums the 8 partials and
    applies gates + b2.

Self-contained: hardcodes all shapes; only imports concourse (system lib).
"""

import os

os.environ.setdefault("JAX_PLATFORMS", "")

import numpy as np
import ml_dtypes

import concourse.bacc as bacc
import concourse.mybir as mybir
import concourse.tile as tile
from concourse.bass_utils import run_bass_kernel_spmd

F8 = ml_dtypes.float8_e4m3

P = 128
D = 1024  # embed dim
H = 4096  # hidden dim
E = 8  # experts
TOPK = 2
NCORES = 8
HS = H // NCORES  # 512: hidden slice per core
KD = D // P  # 8: k-tiles over embed (GEMM1 contraction)
KH = HS // P  # 4: h-tiles in the local slice (GEMM2 contraction)
DT = D // P  # 8: output d-tiles (GEMM2 output)
CW = 512  # chunk width (tokens per moving block; one PSUM bank of fp32)
SH = 32.0  # 2**5 fixed scale for hid in fp8

f32 = mybir.dt.float32
bf16 = mybir.dt.bfloat16
f8 = mybir.dt.float8e4
DR = mybir.MatmulPerfMode.DoubleRow
RELU = mybir.ActivationFunctionType.Relu
MULT = mybir.AluOpType.mult
SUB = mybir.AluOpType.subtract

_compiled = {}
LAST_RESULT = None  # BassKernelResults of the most recent run (for test harness)


def _g1(nc, ps1, chunk, xs, w1h, w1l, b1s, t_p, hs):
    """GEMM1 for one chunk + eviction/split of its hid slice."""
    (ci, e, off, w, s1) = chunk
    for hm in range(KH):
        pt = ps1.tile([P, CW], f32, tag="ps1", name=f"ps1_{ci}_{hm}")
        for k in range(KD):
            nc.tensor.matmul(
                pt[:, :w],
                w1h[:, hm, k].unsqueeze(1).broadcast_to([P, 2, P]),
                xs[:, k, :, :w],
                start=(k == 0),
                stop=False,
                perf_mode=DR,
            )
        for j in range(KD // 2):
            nc.tensor.matmul(
                pt[:, :w],
                w1l[:, hm, j],
                xs[:, 2 * j : 2 * j + 2, 0, :w],
                start=False,
                stop=(j == KD // 2 - 1),
                perf_mode=DR,
            )
        # t = relu(psum*s1 + b1) in bf16, then split into fp8 hi/lo slots
        t = t_p.tile([P, CW], bf16, tag="t", name=f"t_{ci}_{hm}")
        nc.scalar.activation(
            t[:, :w], pt[:, :w], RELU,
            bias=b1s[:, KH * e + hm : KH * e + hm + 1],
            scale=s1,
        )
        nc.gpsimd.tensor_copy(hs[:, hm, 0, :w], t[:, :w])
        nc.vector.scalar_tensor_tensor(
            hs[:, hm, 1, :w], t[:, :w], 1.0, hs[:, hm, 0, :w],
            op0=MULT, op1=SUB,
        )


def _g2(nc, ps2, chunk, hs, w2h, w2l, ob):
    """GEMM2 for one chunk: 8 d-tiles of partial output."""
    (ci, e, off, w, s1) = chunk
    for dt in range(DT):
        pt = ps2.tile([P, CW], f32, tag="ps2", name=f"ps2_{ci}_{dt}")
        for k in range(KH):
            nc.tensor.matmul(
                pt[:, :w],
                w2h[:, dt, k].unsqueeze(1).broadcast_to([P, 2, P]),
                hs[:, k, :, :w],
                start=(k == 0),
                stop=False,
                perf_mode=DR,
            )
        for j in range(KH // 2):
            nc.tensor.matmul(
                pt[:, :w],
                w2l[:, dt, j],
                hs[:, 2 * j : 2 * j + 2, 0, :w],
                start=False,
                stop=(j == KH // 2 - 1),
                perf_mode=DR,
            )
        if dt % 2 == 0:
            nc.vector.tensor_copy(ob[:, dt, :w], pt[:, :w])
        else:
            nc.scalar.copy(ob[:, dt, :w], pt[:, :w])


def _build(chunks, npp):
    """Per-core SPMD program.

    chunks: list of (ci, expert, pair-offset, width, act_scale) covering
    [0, npp).
    """
    key = (npp, tuple(c[1:] for c in chunks))
    if key in _compiled:
        return _compiled[key]

    nc = bacc.Bacc(None, target_bir_lowering=False)
    xs_d = nc.dram_tensor("xs", [P, KD, 2, npp], f8, kind="ExternalInput")
    w1h_d = nc.dram_tensor("w1h", [E, P, KH, KD, P], f8, kind="ExternalInput")
    w1l_d = nc.dram_tensor("w1l", [E, P, KH, KD // 2, 2, P], f8, kind="ExternalInput")
    w2h_d = nc.dram_tensor("w2h", [E, P, DT, KH, P], f8, kind="ExternalInput")
    w2l_d = nc.dram_tensor("w2l", [E, P, DT, KH // 2, 2, P], f8, kind="ExternalInput")
    b1_d = nc.dram_tensor("b1", [P, E * KH], f32, kind="ExternalInput")
    out_d = nc.dram_tensor("out", [P, DT, npp], bf16, kind="ExternalOutput")

    n = len(chunks)

    with tile.TileContext(nc) as tc:
        with (
            tc.tile_pool(name="xs_p", bufs=3) as xs_p,
            tc.tile_pool(name="w_p", bufs=3) as w_p,
            tc.tile_pool(name="t_p", bufs=4) as t_p,
            tc.tile_pool(name="hs_p", bufs=3) as hs_p,
            tc.tile_pool(name="ob_p", bufs=3) as ob_p,
            tc.tile_pool(name="b1_p", bufs=1) as b1_p,
            tc.tile_pool(name="ps1", bufs=4, space="PSUM") as ps1,
            tc.tile_pool(name="ps2", bufs=4, space="PSUM") as ps2,
        ):

            def load_w1(e):
                w1h = w_p.tile([P, KH, KD, P], f8, tag="w1h", name=f"w1h_{e}")
                w1l = w_p.tile([P, KH, KD // 2, 2, P], f8, tag="w1l", name=f"w1l_{e}")
                nc.sync.dma_start(w1h[:], w1h_d[e])
                nc.sync.dma_start(w1l[:], w1l_d[e])
                return (w1h, w1l)

            def load_w2(e):
                w2h = w_p.tile([P, DT, KH, P], f8, tag="w2h", name=f"w2h_{e}")
                w2l = w_p.tile([P, DT, KH // 2, 2, P], f8, tag="w2l", name=f"w2l_{e}")
                nc.sync.dma_start(w2h[:], w2h_d[e])
                nc.sync.dma_start(w2l[:], w2l_d[e])
                return (w2h, w2l)

            def load_xs(c):
                (ci, e, off, w, s1) = c
                xs = xs_p.tile([P, KD, 2, CW], f8, tag="xs", name=f"xs_{ci}")
                nc.sync.dma_start(xs[:, :, :, :w], xs_d[:, :, :, off : off + w])
                return xs

            # prologue issue order: the bytes GEMM1(chunk 0) needs come first
            b1s = b1_p.tile([P, E * KH], f32, name="b1s")
            nc.sync.dma_start(b1s[:], b1_d[:])
            e0 = chunks[0][1]
            xmap = {0: load_xs(chunks[0])}
            wmap = {e0: load_w1(e0)}
            if n > 1:
                xmap[1] = load_xs(chunks[1])
            wmap[e0] = wmap[e0] + load_w2(e0)
            for c in chunks[1:3]:
                if c[1] not in wmap:
                    wmap[c[1]] = load_w1(c[1]) + load_w2(c[1])
                if c[0] not in xmap:
                    xmap[c[0]] = load_xs(c)

            hsm = {}

            def emit_g1(c):
                (ci, e, off, w, s1) = c
                hs = hs_p.tile([P, KH, 2, CW], f8, tag="hs", name=f"hs_{ci}")
                hsm[ci] = hs
                _g1(nc, ps1, c, xmap.pop(ci), wmap[e][0], wmap[e][1], b1s, t_p, hs)

            def emit_g2(c):
                (ci, e, off, w, s1) = c
                ob = ob_p.tile([P, DT, CW], bf16, tag="ob", name=f"ob_{ci}")
                _g2(nc, ps2, c, hsm.pop(ci), wmap[e][2], wmap[e][3], ob)
                nc.scalar.dma_start(out_d[:, :, off : off + w], ob[:, :, :w])

            # software pipeline: G1(i+1) is emitted before G2(i) so the PE
            # never waits on the ACT/Pool/DVE hid-split chain.
            emit_g1(chunks[0])
            for i in range(n):
                if i + 2 < n:
                    c2 = chunks[i + 2]
                    xmap[c2[0]] = load_xs(c2)
                    if c2[1] not in wmap:
                        wmap[c2[1]] = load_w1(c2[1]) + load_w2(c2[1])
                if i + 1 < n:
                    emit_g1(chunks[i + 1])
                emit_g2(chunks[i])
                ce = chunks[i][1]
                if i + 1 == n or chunks[i + 1][1] != ce:
                    del wmap[ce]

    nc.compile()
    _compiled[key] = nc
    return nc


def _quant_split(a):
    """e4m3 hi/lo split of a pre-scaled float32 array."""
    hi = a.astype(F8)
    lo = (a - hi.astype(np.float32)).astype(F8)
    return hi, lo


def _pow2_scale(maxval, target=160.0):
    return float(2.0 ** np.floor(np.log2(target / maxval)))


def kernel(x, Wg, bg, W1, b1, W2, b2):
    global LAST_RESULT
    x = np.ascontiguousarray(x, dtype=np.float32)
    B, S, d = x.shape
    assert d == D
    T = B * S
    xf = x.reshape(T, d)

    # ---- Host gating/routing (fp64) ----
    logits = xf.astype(np.float64) @ np.asarray(Wg, np.float64) + np.asarray(
        bg, np.float64
    )
    mx = logits.max(axis=1, keepdims=True)
    ex = np.exp(logits - mx)
    probs = ex / ex.sum(axis=1, keepdims=True)
    order = np.argsort(-logits, axis=1, kind="stable")  # ties -> lower index
    top = order[:, :TOPK]  # [T, 2]
    gsel = np.take_along_axis(probs, top, axis=1).astype(np.float32)

    toks, gates = [], []
    for e in range(E):
        pos = top == e  # [T, 2]
        sel = pos.any(axis=1)
        toks.append(np.nonzero(sel)[0])
        gates.append((gsel * pos).sum(axis=1)[sel].astype(np.float32))

    # pair layout: expert-major, each expert's count padded to a multiple
    # of 16 (DoubleRow AP alignment)
    cnt = [len(t) for t in toks]
    cnt16 = [-(-c // 16) * 16 for c in cnt]
    offs = np.concatenate([[0], np.cumsum(cnt16)]).astype(np.int64)
    npp = int(offs[-1])
    pair_tok = np.zeros(npp, np.int64)
    for e in range(E):
        pair_tok[offs[e] : offs[e] + cnt[e]] = toks[e]

    # ---- scales (powers of 2; lossless to apply) ----
    sx = _pow2_scale(np.abs(xf).max())
    sw1 = _pow2_scale(np.abs(W1).max())
    sw2 = _pow2_scale(np.abs(W2).max())
    s1 = SH / (sx * sw1)  # ACT scale: psum1 -> hid*SH
    inv_out = 1.0 / (SH * sw2)

    # chunk widths: prefer full 512s (512B DMA descriptors); keep every
    # chunk >= 256 so the next chunk's GEMM1 always covers the hid-split
    # chain latency (split a trailing 512+r when the remainder is small)
    def plan_widths(tot):
        n512, r = divmod(tot, CW)
        if r == 0:
            ws = [CW] * n512
        elif r >= 256 or n512 == 0:
            ws = [CW] * n512 + [r]
        else:
            half = (CW + r) // 2 // 16 * 16
            ws = [CW] * (n512 - 1) + [half, CW + r - half]
        return ws

    widths = []
    for e in range(E):
        widths.append(plan_widths(cnt16[e]))
    # split the very last chunk so the final GEMM2 is covered by a GEMM1
    lw = widths[-1][-1]
    if lw >= 256:
        widths[-1] = widths[-1][:-1] + [lw - 128, 128]

    chunks = []
    ci = 0
    for e in range(E):
        off = int(offs[e])
        for w in widths[e]:
            chunks.append((ci, e, off, w, s1))
            ci += 1
            off += w

    # ---- x: gather pairs, scale, split, arrange [P, KD, 2, npp] ----
    xg = xf[pair_tok] * sx
    xh, xl = _quant_split(xg)
    xs_host = np.empty((P, KD, 2, npp), F8)
    xs_host[:, :, 0, :] = xh.reshape(npp, KD, P).transpose(2, 1, 0)
    xs_host[:, :, 1, :] = xl.reshape(npp, KD, P).transpose(2, 1, 0)

    # ---- per-core weight shards ----
    W1f = np.asarray(W1, np.float32) * sw1
    W2f = np.asarray(W2, np.float32) * sw2
    b1f = np.asarray(b1, np.float32) * SH
    core_maps = []
    for c in range(NCORES):
        sl = slice(c * HS, (c + 1) * HS)
        w1hi, w1lo = _quant_split(W1f[:, :, sl])  # [E, D, HS]
        w2hi, w2lo = _quant_split(W2f[:, sl, :])  # [E, HS, D]
        # GEMM1 stationary: [e, p(d-in-k), hm, k, j(h-in-hm)] (hi, no dup —
        # the device broadcasts the DoubleRow slot pair with a stride-0 AP)
        a = w1hi.reshape(E, KD, P, KH, P).transpose(0, 2, 3, 1, 4)  # [E,p,hm,k,j]
        w1h_host = np.ascontiguousarray(a)
        bl = w1lo.reshape(E, KD, P, KH, P).transpose(0, 2, 3, 1, 4)
        w1l_host = np.ascontiguousarray(bl.reshape(E, P, KH, KD // 2, 2, P))
        # GEMM2 stationary: [e, p(h-in-k), dt, k, j(d-in-dt)]
        a2 = w2hi.reshape(E, KH, P, DT, P).transpose(0, 2, 3, 1, 4)  # [E,p,dt,k,j]
        w2h_host = np.ascontiguousarray(a2)
        b2l = w2lo.reshape(E, KH, P, DT, P).transpose(0, 2, 3, 1, 4)
        w2l_host = np.ascontiguousarray(b2l.reshape(E, P, DT, KH // 2, 2, P))
        b1_host = np.ascontiguousarray(
            b1f[:, sl].reshape(E, KH, P).transpose(2, 0, 1).reshape(P, E * KH)
        )
        core_maps.append(
            {
                "xs": xs_host,
                "w1h": w1h_host,
                "w1l": w1l_host,
                "w2h": w2h_host,
                "w2l": w2l_host,
                "b1": b1_host,
            }
        )

    nc = _build(chunks, npp)
    res = run_bass_kernel_spmd(nc, core_maps, core_ids=list(range(NCORES)))
    LAST_RESULT = res

    # ---- combine partials on host ----
    total = np.zeros((P, DT, npp), np.float32)
    for c in range(NCORES):
        total += np.asarray(res.results[c]["out"]).astype(np.float32)
    # [p, dt, pair] -> [pair, dt*128=d]
    ytot = total.transpose(2, 1, 0).reshape(npp, D) * inv_out

    out = np.zeros((T, D), np.float32)
    b2f = np.asarray(b2, np.float32)
    for e in range(E):
        if cnt[e] == 0:
            continue
        ye = ytot[offs[e] : offs[e] + cnt[e]]
        out[toks[e]] += gates[e][:, None] * (ye + b2f[e])
    return out.reshape(B, S, D)


# revision 30
# speedup vs baseline: 1.5678x; 1.0218x over previous
"""MoE (top-2 of 8 experts, d=1024, h=4096) on 8 Trainium2 NeuronCores.

Strategy (hidden-dim sharding + fp8 DoubleRow split matmuls):
  - Host: gating in fp64 (tie margins ~1e-5 >> fp32 noise, so top-2 matches
    the reference), token-pair list ordered by expert (each expert's count
    padded to a multiple of 16), power-of-2 scaling + e4m3 hi/lo splitting
    of x and all weights.
  - Each core processes ALL 16384 token-expert pairs but only a 512-wide
    slice of the hidden dim (h-shard) of every expert -> perfect load
    balance (zero capacity padding), identical SPMD program on all cores.
  - GEMM1 (x @ W1_slice) and GEMM2 (hid_slice @ W2_slice) both run as fp8
    DoubleRow matmuls (two independent 128-deep contractions summed per
    instruction at 0.5 cycles/row). The 3-term split
        x @ W ~= Wh.T@(xh+xl) + Wl.T@xh
    costs 0.75x of one bf16 matmul at ~1e-3 accuracy:
      hi pass: lhsT slots (Wh, Wh) x rhs slots (xh, xl)   [1 DR / k-tile]
      lo pass: lhsT slots (Wl_2j, Wl_2j+1) x rhs (xh_2j, xh_2j+1)
                                                          [1 DR / 2 k-tiles]
  - hid stays in SBUF: ACT evicts psum1 -> t = relu(scale*psum+b1) bf16;
    gpsimd casts hh = fp8(t); DVE computes hl = fp8(t - hh). GEMM2 reads
    (hh, hl) slots. psum2 evicted to bf16 (DVE/ACT alternating) and DMAd
    out as partial sums over the h-shard; host sums the 8 partials and
    applies gates + b2.

Self-contained: hardcodes all shapes; only imports concourse (system lib).
"""

import os

os.environ.setdefault("JAX_PLATFORMS", "")

import numpy as np
import ml_dtypes

import concourse.bacc as bacc
import concourse.mybir as mybir
import concourse.tile as tile
from concourse.bass_utils import run_bass_kernel_spmd

F8 = ml_dtypes.float8_e4m3

P = 128
D = 1024  # embed dim
H = 4096  # hidden dim
E = 8  # experts
TOPK = 2
NCORES = 8
HS = H // NCORES  # 512: hidden slice per core
KD = D // P  # 8: k-tiles over embed (GEMM1 contraction)
KH = HS // P  # 4: h-tiles in the local slice (GEMM2 contraction)
DT = D // P  # 8: output d-tiles (GEMM2 output)
CW = 512  # chunk width (tokens per moving block; one PSUM bank of fp32)
SH = 32.0  # 2**5 fixed scale for hid in fp8

f32 = mybir.dt.float32
bf16 = mybir.dt.bfloat16
f8 = mybir.dt.float8e4
DR = mybir.MatmulPerfMode.DoubleRow
RELU = mybir.ActivationFunctionType.Relu
MULT = mybir.AluOpType.mult
SUB = mybir.AluOpType.subtract

_compiled = {}
LAST_RESULT = None  # BassKernelResults of the most recent run (for test harness)


def _g1(nc, ps1, chunk, xs, w1h, w1l, b1s, t_p, hs, tail=False, kmajor=False):
    """GEMM1 for one chunk + eviction/split of its hid slice.

    kmajor (first chunk): sweep k outer / hm inner so the earliest matmuls
    only need the first k-pieces of the streaming x load.
    """
    (ci, e, off, w, s1) = chunk
    pts = [ps1.tile([P, CW], f32, tag="ps1", name=f"ps1_{ci}_{hm}")
           for hm in range(KH)]

    def hi(hm, k):
        nc.tensor.matmul(
            pts[hm][:, :w],
            w1h[:, hm, k].unsqueeze(1).broadcast_to([P, 2, P]),
            xs[:, k, :, :w],
            start=(k == 0),
            stop=False,
            perf_mode=DR,
        )

    def lo(hm, j):
        nc.tensor.matmul(
            pts[hm][:, :w],
            w1l[:, hm, j],
            xs[:, 2 * j : 2 * j + 2, 0, :w],
            start=False,
            stop=(j == KD // 2 - 1),
            perf_mode=DR,
        )

    def evict(hm):
        # t = relu(psum*s1 + b1) in bf16, then split into fp8 hi/lo slots
        t = t_p.tile([P, CW], bf16, tag="t", name=f"t_{ci}_{hm}")
        nc.scalar.activation(
            t[:, :w], pts[hm][:, :w], RELU,
            bias=b1s[:, KH * e + hm : KH * e + hm + 1],
            scale=s1,
        )
        nc.gpsimd.tensor_copy(hs[:, hm, 0, :w], t[:, :w])
        nc.vector.scalar_tensor_tensor(
            hs[:, hm, 1, :w], t[:, :w], 1.0, hs[:, hm, 0, :w],
            op0=MULT, op1=SUB,
        )

    if kmajor:
        for k in range(KD):
            for hm in range(KH):
                hi(hm, k)
        for j in range(KD // 2):
            for hm in range(KH):
                lo(hm, j)
        for hm in range(KH):
            evict(hm)
    else:
        for hm in range(KH):
            for k in range(KD):
                hi(hm, k)
            for j in range(KD // 2):
                lo(hm, j)
            evict(hm)


def _g2(nc, ps2, chunk, hs, w2h, w2l, ob, store=None):
    """GEMM2 for one chunk: 8 d-tiles of partial output.

    store(lo_dt, hi_dt): issue the output store for a d-tile range as soon
    as its evictions are emitted (halves the trailing store latency).
    """
    (ci, e, off, w, s1) = chunk
    for dt in range(DT):
        pt = ps2.tile([P, CW], f32, tag="ps2", name=f"ps2_{ci}_{dt}")
        for k in range(KH):
            nc.tensor.matmul(
                pt[:, :w],
                w2h[:, dt, k].unsqueeze(1).broadcast_to([P, 2, P]),
                hs[:, k, :, :w],
                start=(k == 0),
                stop=False,
                perf_mode=DR,
            )
        for j in range(KH // 2):
            nc.tensor.matmul(
                pt[:, :w],
                w2l[:, dt, j],
                hs[:, 2 * j : 2 * j + 2, 0, :w],
                start=False,
                stop=(j == KH // 2 - 1),
                perf_mode=DR,
            )
        if dt % 2 == 0:
            nc.vector.tensor_copy(ob[:, dt, :w], pt[:, :w])
        else:
            nc.scalar.copy(ob[:, dt, :w], pt[:, :w])
        if store is not None and (dt + 1) % store[1] == 0:
            store[0](dt + 1 - store[1], dt + 1)


def _build(chunks, npp):
    """Per-core SPMD program.

    chunks: list of (ci, expert, pair-offset, width, act_scale) covering
    [0, npp).
    """
    key = (npp, tuple(c[1:] for c in chunks))
    if key in _compiled:
        return _compiled[key]

    nc = bacc.Bacc(None, target_bir_lowering=False)
    xs_d = nc.dram_tensor("xs", [P, KD, 2, npp], f8, kind="ExternalInput")
    w1h_d = nc.dram_tensor("w1h", [E, P, KH, KD, P], f8, kind="ExternalInput")
    w1l_d = nc.dram_tensor("w1l", [E, P, KH, KD // 2, 2, P], f8, kind="ExternalInput")
    w2h_d = nc.dram_tensor("w2h", [E, P, DT, KH, P], f8, kind="ExternalInput")
    w2l_d = nc.dram_tensor("w2l", [E, P, DT, KH // 2, 2, P], f8, kind="ExternalInput")
    b1_d = nc.dram_tensor("b1", [P, E * KH], f32, kind="ExternalInput")
    out_d = nc.dram_tensor("out", [P, DT, npp], bf16, kind="ExternalOutput")

    n = len(chunks)

    with tile.TileContext(nc) as tc:
        with (
            tc.tile_pool(name="xs_p", bufs=3) as xs_p,
            tc.tile_pool(name="w_p", bufs=3) as w_p,
            tc.tile_pool(name="t_p", bufs=4) as t_p,
            tc.tile_pool(name="hs_p", bufs=3) as hs_p,
            tc.tile_pool(name="ob_p", bufs=3) as ob_p,
            tc.tile_pool(name="b1_p", bufs=1) as b1_p,
            tc.tile_pool(name="ps1", bufs=4, space="PSUM") as ps1,
            tc.tile_pool(name="ps2", bufs=4, space="PSUM") as ps2,
        ):

            def load_w1(e):
                w1h = w_p.tile([P, KH, KD, P], f8, tag="w1h", name=f"w1h_{e}")
                w1l = w_p.tile([P, KH, KD // 2, 2, P], f8, tag="w1l", name=f"w1l_{e}")
                nc.sync.dma_start(w1h[:], w1h_d[e])
                nc.sync.dma_start(w1l[:], w1l_d[e])
                return (w1h, w1l)

            def load_w2(e):
                w2h = w_p.tile([P, DT, KH, P], f8, tag="w2h", name=f"w2h_{e}")
                w2l = w_p.tile([P, DT, KH // 2, 2, P], f8, tag="w2l", name=f"w2l_{e}")
                nc.sync.dma_start(w2h[:], w2h_d[e])
                nc.sync.dma_start(w2l[:], w2l_d[e])
                return (w2h, w2l)

            def load_xs(c):
                (ci, e, off, w, s1) = c
                xs = xs_p.tile([P, KD, 2, CW], f8, tag="xs", name=f"xs_{ci}")
                nc.sync.dma_start(xs[:, :, :, :w], xs_d[:, :, :, off : off + w])
                return xs

            # prologue issue order: the bytes GEMM1(chunk 0, hm 0, k<4)
            # needs come first, in fine-grained pieces
            e0 = chunks[0][1]
            w0 = chunks[0][3]
            w1h0 = w_p.tile([P, KH, KD, P], f8, tag="w1h", name=f"w1h_{e0}")
            nc.sync.dma_start(w1h0[:, : KH // 2], w1h_d[e0, :, : KH // 2])
            xs0 = xs_p.tile([P, KD, 2, CW], f8, tag="xs", name="xs_0")
            nc.sync.dma_start(
                xs0[:, : KD // 2, :, :w0], xs_d[:, : KD // 2, :, :w0]
            )
            nc.sync.dma_start(
                xs0[:, KD // 2 :, :, :w0], xs_d[:, KD // 2 :, :, :w0]
            )
            w1l0 = w_p.tile([P, KH, KD // 2, 2, P], f8, tag="w1l", name=f"w1l_{e0}")
            nc.sync.dma_start(w1l0[:, : KH // 2], w1l_d[e0, :, : KH // 2])
            b1s = b1_p.tile([P, E * KH], f32, name="b1s")
            nc.sync.dma_start(b1s[:], b1_d[:])
            nc.sync.dma_start(w1h0[:, KH // 2 :], w1h_d[e0, :, KH // 2 :])
            nc.sync.dma_start(w1l0[:, KH // 2 :], w1l_d[e0, :, KH // 2 :])
            xmap = {0: xs0}
            wmap = {e0: (w1h0, w1l0)}
            if n > 1:
                xmap[1] = load_xs(chunks[1])
            wmap[e0] = wmap[e0] + load_w2(e0)
            for c in chunks[1:3]:
                if c[1] not in wmap:
                    wmap[c[1]] = load_w1(c[1]) + load_w2(c[1])
                if c[0] not in xmap:
                    xmap[c[0]] = load_xs(c)

            hsm = {}

            def emit_g1(c, tail=False):
                (ci, e, off, w, s1) = c
                hs = hs_p.tile([P, KH, 2, CW], f8, tag="hs", name=f"hs_{ci}")
                hsm[ci] = hs
                _g1(nc, ps1, c, xmap.pop(ci), wmap[e][0], wmap[e][1], b1s,
                    t_p, hs, tail=tail)

            def emit_g2(c, tail=False):
                (ci, e, off, w, s1) = c
                ob = ob_p.tile([P, DT, CW], bf16, tag="ob", name=f"ob_{ci}")
                ring = nc.sync if tail else nc.scalar  # tail: idle SP ring

                def st(lo, hi):
                    ring.dma_start(
                        out_d[:, lo:hi, off : off + w], ob[:, lo:hi, :w]
                    )

                _g2(nc, ps2, c, hsm.pop(ci), wmap[e][2], wmap[e][3], ob,
                    store=(st, DT // 2 if tail else DT))

            # software pipeline: G1(i+1) is emitted before G2(i) so the PE
            # never waits on the ACT/Pool/DVE hid-split chain; at the tail
            # the last G1 is emitted two steps early (depth-2) since the
            # final chunks are narrow.
            emit_g1(chunks[0])
            g1p = 1
            for i in range(n):
                if i + 2 < n:
                    c2 = chunks[i + 2]
                    xmap[c2[0]] = load_xs(c2)
                    if c2[1] not in wmap:
                        wmap[c2[1]] = load_w1(c2[1]) + load_w2(c2[1])
                tgt = min(n - 1, i + 1 if i != n - 3 else n - 1)
                while g1p <= tgt:
                    emit_g1(chunks[g1p], tail=(g1p >= n - 2))
                    g1p += 1
                emit_g2(chunks[i], tail=(i >= n - 3))
                ce = chunks[i][1]
                if i + 1 == n or chunks[i + 1][1] != ce:
                    del wmap[ce]

    nc.compile()
    _compiled[key] = nc
    return nc


def _quant_split(a):
    """e4m3 hi/lo split of a pre-scaled float32 array."""
    hi = a.astype(F8)
    lo = (a - hi.astype(np.float32)).astype(F8)
    return hi, lo


def _pow2_scale(maxval, target=160.0):
    return float(2.0 ** np.floor(np.log2(target / maxval)))


def kernel(x, Wg, bg, W1, b1, W2, b2):
    global LAST_RESULT
    x = np.ascontiguousarray(x, dtype=np.float32)
    B, S, d = x.shape
    assert d == D
    T = B * S
    xf = x.reshape(T, d)

    # ---- Host gating/routing (fp64) ----
    logits = xf.astype(np.float64) @ np.asarray(Wg, np.float64) + np.asarray(
        bg, np.float64
    )
    mx = logits.max(axis=1, keepdims=True)
    ex = np.exp(logits - mx)
    probs = ex / ex.sum(axis=1, keepdims=True)
    order = np.argsort(-logits, axis=1, kind="stable")  # ties -> lower index
    top = order[:, :TOPK]  # [T, 2]
    gsel = np.take_along_axis(probs, top, axis=1).astype(np.float32)

    toks, gates = [], []
    for e in range(E):
        pos = top == e  # [T, 2]
        sel = pos.any(axis=1)
        toks.append(np.nonzero(sel)[0])
        gates.append((gsel * pos).sum(axis=1)[sel].astype(np.float32))

    # pair layout: expert-major, each expert's count padded to a multiple
    # of 16 (DoubleRow AP alignment)
    cnt = [len(t) for t in toks]
    cnt16 = [-(-c // 16) * 16 for c in cnt]
    offs = np.concatenate([[0], np.cumsum(cnt16)]).astype(np.int64)
    npp = int(offs[-1])
    pair_tok = np.zeros(npp, np.int64)
    for e in range(E):
        pair_tok[offs[e] : offs[e] + cnt[e]] = toks[e]

    # ---- scales (powers of 2; lossless to apply) ----
    sx = _pow2_scale(np.abs(xf).max())
    sw1 = _pow2_scale(np.abs(W1).max())
    sw2 = _pow2_scale(np.abs(W2).max())
    s1 = SH / (sx * sw1)  # ACT scale: psum1 -> hid*SH
    inv_out = 1.0 / (SH * sw2)

    # chunk widths: prefer full 512s (512B DMA descriptors); keep every
    # chunk >= 256 so the next chunk's GEMM1 always covers the hid-split
    # chain latency (split a trailing 512+r when the remainder is small)
    def plan_widths(tot):
        n512, r = divmod(tot, CW)
        if r == 0:
            ws = [CW] * n512
        elif r >= 256 or n512 == 0:
            ws = [CW] * n512 + [r]
        else:
            half = (CW + r) // 2 // 16 * 16
            ws = [CW] * (n512 - 1) + [half, CW + r - half]
        return ws

    widths = []
    for e in range(E):
        widths.append(plan_widths(cnt16[e]))
    # split the very last chunk so the final GEMM2 is covered by a GEMM1
    lw = widths[-1][-1]
    if lw >= 256:
        widths[-1] = widths[-1][:-1] + [lw - 128, 128]

    chunks = []
    ci = 0
    for e in range(E):
        off = int(offs[e])
        for w in widths[e]:
            chunks.append((ci, e, off, w, s1))
            ci += 1
            off += w

    # ---- x: gather pairs, scale, split, arrange [P, KD, 2, npp] ----
    xg = xf[pair_tok] * sx
    xh, xl = _quant_split(xg)
    xs_host = np.empty((P, KD, 2, npp), F8)
    xs_host[:, :, 0, :] = xh.reshape(npp, KD, P).transpose(2, 1, 0)
    xs_host[:, :, 1, :] = xl.reshape(npp, KD, P).transpose(2, 1, 0)

    # ---- per-core weight shards ----
    W1f = np.asarray(W1, np.float32) * sw1
    W2f = np.asarray(W2, np.float32) * sw2
    b1f = np.asarray(b1, np.float32) * SH
    core_maps = []
    for c in range(NCORES):
        sl = slice(c * HS, (c + 1) * HS)
        w1hi, w1lo = _quant_split(W1f[:, :, sl])  # [E, D, HS]
        w2hi, w2lo = _quant_split(W2f[:, sl, :])  # [E, HS, D]
        # GEMM1 stationary: [e, p(d-in-k), hm, k, j(h-in-hm)] (hi, no dup —
        # the device broadcasts the DoubleRow slot pair with a stride-0 AP)
        a = w1hi.reshape(E, KD, P, KH, P).transpose(0, 2, 3, 1, 4)  # [E,p,hm,k,j]
        w1h_host = np.ascontiguousarray(a)
        bl = w1lo.reshape(E, KD, P, KH, P).transpose(0, 2, 3, 1, 4)
        w1l_host = np.ascontiguousarray(bl.reshape(E, P, KH, KD // 2, 2, P))
        # GEMM2 stationary: [e, p(h-in-k), dt, k, j(d-in-dt)]
        a2 = w2hi.reshape(E, KH, P, DT, P).transpose(0, 2, 3, 1, 4)  # [E,p,dt,k,j]
        w2h_host = np.ascontiguousarray(a2)
        b2l = w2lo.reshape(E, KH, P, DT, P).transpose(0, 2, 3, 1, 4)
        w2l_host = np.ascontiguousarray(b2l.reshape(E, P, DT, KH // 2, 2, P))
        b1_host = np.ascontiguousarray(
            b1f[:, sl].reshape(E, KH, P).transpose(2, 0, 1).reshape(P, E * KH)
        )
        core_maps.append(
            {
                "xs": xs_host,
                "w1h": w1h_host,
                "w1l": w1l_host,
                "w2h": w2h_host,
                "w2l": w2l_host,
                "b1": b1_host,
            }
        )

    nc = _build(chunks, npp)
    res = run_bass_kernel_spmd(nc, core_maps, core_ids=list(range(NCORES)))
    LAST_RESULT = res

    # ---- combine partials on host ----
    total = np.zeros((P, DT, npp), np.float32)
    for c in range(NCORES):
        total += np.asarray(res.results[c]["out"]).astype(np.float32)
    # [p, dt, pair] -> [pair, dt*128=d]
    ytot = total.transpose(2, 1, 0).reshape(npp, D) * inv_out

    out = np.zeros((T, D), np.float32)
    b2f = np.asarray(b2, np.float32)
    for e in range(E):
        if cnt[e] == 0:
            continue
        ye = ytot[offs[e] : offs[e] + cnt[e]]
        out[toks[e]] += gates[e][:, None] * (ye + b2f[e])
    return out.reshape(B, S, D)


# revision 33
# speedup vs baseline: 1.5735x; 1.0036x over previous
"""MoE (top-2 of 8 experts, d=1024, h=4096) on 8 Trainium2 NeuronCores.

Strategy (hidden-dim sharding + fp8 DoubleRow split matmuls):
  - Host: gating in fp64 (tie margins ~1e-5 >> fp32 noise, so top-2 matches
    the reference), token-pair list ordered by expert (each expert's count
    padded to a multiple of 16), power-of-2 scaling + e4m3 hi/lo splitting
    of x and all weights.
  - Each core processes ALL 16384 token-expert pairs but only a 512-wide
    slice of the hidden dim (h-shard) of every expert -> perfect load
    balance (zero capacity padding), identical SPMD program on all cores.
  - GEMM1 (x @ W1_slice) and GEMM2 (hid_slice @ W2_slice) both run as fp8
    DoubleRow matmuls (two independent 128-deep contractions summed per
    instruction at 0.5 cycles/row). The 3-term split
        x @ W ~= Wh.T@(xh+xl) + Wl.T@xh
    costs 0.75x of one bf16 matmul at ~1e-3 accuracy:
      hi pass: lhsT slots (Wh, Wh) x rhs slots (xh, xl)   [1 DR / k-tile]
      lo pass: lhsT slots (Wl_2j, Wl_2j+1) x rhs (xh_2j, xh_2j+1)
                                                          [1 DR / 2 k-tiles]
  - hid stays in SBUF: ACT evicts psum1 -> t = relu(scale*psum+b1) bf16;
    gpsimd casts hh = fp8(t); DVE computes hl = fp8(t - hh). GEMM2 reads
    (hh, hl) slots. psum2 evicted to bf16 (DVE/ACT alternating) and DMAd
    out as partial sums over the h-shard; host sums the 8 partials and
    applies gates + b2.

Self-contained: hardcodes all shapes; only imports concourse (system lib).
"""

import os

os.environ.setdefault("JAX_PLATFORMS", "")

import numpy as np
import ml_dtypes

import concourse.bacc as bacc
import concourse.mybir as mybir
import concourse.tile as tile
from concourse.bass_utils import run_bass_kernel_spmd

F8 = ml_dtypes.float8_e4m3

P = 128
D = 1024  # embed dim
H = 4096  # hidden dim
E = 8  # experts
TOPK = 2
NCORES = 8
HS = H // NCORES  # 512: hidden slice per core
KD = D // P  # 8: k-tiles over embed (GEMM1 contraction)
KH = HS // P  # 4: h-tiles in the local slice (GEMM2 contraction)
DT = D // P  # 8: output d-tiles (GEMM2 output)
CW = 512  # chunk width (tokens per moving block; one PSUM bank of fp32)
SH = 32.0  # 2**5 fixed scale for hid in fp8

f32 = mybir.dt.float32
bf16 = mybir.dt.bfloat16
f8 = mybir.dt.float8e4
DR = mybir.MatmulPerfMode.DoubleRow
RELU = mybir.ActivationFunctionType.Relu
MULT = mybir.AluOpType.mult
SUB = mybir.AluOpType.subtract

_compiled = {}
LAST_RESULT = None  # BassKernelResults of the most recent run (for test harness)


def _g1(nc, ps1, chunk, xs, w1h, w1l, b1s, t_p, hs, tail=False, kmajor=False):
    """GEMM1 for one chunk + eviction/split of its hid slice.

    kmajor (first chunk): sweep k outer / hm inner so the earliest matmuls
    only need the first k-pieces of the streaming x load.
    """
    (ci, e, off, w, s1) = chunk
    pts = [ps1.tile([P, CW], f32, tag="ps1", name=f"ps1_{ci}_{hm}")
           for hm in range(KH)]

    def hi(hm, k):
        nc.tensor.matmul(
            pts[hm][:, :w],
            w1h[:, hm, k].unsqueeze(1).broadcast_to([P, 2, P]),
            xs[:, k, :, :w],
            start=(k == 0),
            stop=False,
            perf_mode=DR,
        )

    def lo(hm, j):
        nc.tensor.matmul(
            pts[hm][:, :w],
            w1l[:, hm, j],
            xs[:, 2 * j : 2 * j + 2, 0, :w],
            start=False,
            stop=(j == KD // 2 - 1),
            perf_mode=DR,
        )

    def evict(hm):
        # t = relu(psum*s1 + b1) in bf16, then split into fp8 hi/lo slots
        t = t_p.tile([P, CW], bf16, tag="t", name=f"t_{ci}_{hm}")
        nc.scalar.activation(
            t[:, :w], pts[hm][:, :w], RELU,
            bias=b1s[:, KH * e + hm : KH * e + hm + 1],
            scale=s1,
        )
        nc.gpsimd.tensor_copy(hs[:, hm, 0, :w], t[:, :w])
        nc.vector.scalar_tensor_tensor(
            hs[:, hm, 1, :w], t[:, :w], 1.0, hs[:, hm, 0, :w],
            op0=MULT, op1=SUB,
        )

    if kmajor:
        for k in range(KD):
            for hm in range(KH):
                hi(hm, k)
        for j in range(KD // 2):
            for hm in range(KH):
                lo(hm, j)
        for hm in range(KH):
            evict(hm)
    else:
        for hm in range(KH):
            for k in range(KD):
                hi(hm, k)
            for j in range(KD // 2):
                lo(hm, j)
            evict(hm)


def _g2(nc, ps2, chunk, hs, w2h, w2l, ob, store=None):
    """GEMM2 for one chunk: 8 d-tiles of partial output.

    store(lo_dt, hi_dt): issue the output store for a d-tile range as soon
    as its evictions are emitted (halves the trailing store latency).
    """
    (ci, e, off, w, s1) = chunk
    for dt in range(DT):
        pt = ps2.tile([P, CW], f32, tag="ps2", name=f"ps2_{ci}_{dt}")
        for k in range(KH):
            nc.tensor.matmul(
                pt[:, :w],
                w2h[:, dt, k].unsqueeze(1).broadcast_to([P, 2, P]),
                hs[:, k, :, :w],
                start=(k == 0),
                stop=False,
                perf_mode=DR,
            )
        for j in range(KH // 2):
            nc.tensor.matmul(
                pt[:, :w],
                w2l[:, dt, j],
                hs[:, 2 * j : 2 * j + 2, 0, :w],
                start=False,
                stop=(j == KH // 2 - 1),
                perf_mode=DR,
            )
        if dt % 2 == 0:
            nc.vector.tensor_copy(ob[:, dt, :w], pt[:, :w])
        else:
            nc.scalar.copy(ob[:, dt, :w], pt[:, :w])
        if store is not None and (dt + 1) % store[1] == 0:
            store[0](dt + 1 - store[1], dt + 1)


def _build(chunks, npp):
    """Per-core SPMD program.

    chunks: list of (ci, expert, pair-offset, width, act_scale) covering
    [0, npp).
    """
    key = (npp, tuple(c[1:] for c in chunks))
    if key in _compiled:
        return _compiled[key]

    nc = bacc.Bacc(None, target_bir_lowering=False)
    xs_d = nc.dram_tensor("xs", [P, KD, 2, npp], f8, kind="ExternalInput")
    w1h_d = nc.dram_tensor("w1h", [E, P, KH, KD, P], f8, kind="ExternalInput")
    w1l_d = nc.dram_tensor("w1l", [E, P, KH, KD // 2, 2, P], f8, kind="ExternalInput")
    w2h_d = nc.dram_tensor("w2h", [E, P, DT, KH, P], f8, kind="ExternalInput")
    w2l_d = nc.dram_tensor("w2l", [E, P, DT, KH // 2, 2, P], f8, kind="ExternalInput")
    b1_d = nc.dram_tensor("b1", [P, E * KH], f32, kind="ExternalInput")
    out_d = nc.dram_tensor("out", [P, DT, npp], bf16, kind="ExternalOutput")

    n = len(chunks)

    with tile.TileContext(nc) as tc:
        with (
            tc.tile_pool(name="xs_p", bufs=3) as xs_p,
            tc.tile_pool(name="w_p", bufs=3) as w_p,
            tc.tile_pool(name="t_p", bufs=4) as t_p,
            tc.tile_pool(name="hs_p", bufs=3) as hs_p,
            tc.tile_pool(name="ob_p", bufs=3) as ob_p,
            tc.tile_pool(name="b1_p", bufs=1) as b1_p,
            tc.tile_pool(name="ps1", bufs=4, space="PSUM") as ps1,
            tc.tile_pool(name="ps2", bufs=4, space="PSUM") as ps2,
        ):

            def load_w1(e):
                w1h = w_p.tile([P, KH, KD, P], f8, tag="w1h", name=f"w1h_{e}")
                w1l = w_p.tile([P, KH, KD // 2, 2, P], f8, tag="w1l", name=f"w1l_{e}")
                nc.sync.dma_start(w1h[:], w1h_d[e])
                nc.sync.dma_start(w1l[:], w1l_d[e])
                return (w1h, w1l)

            def load_w2(e):
                w2h = w_p.tile([P, DT, KH, P], f8, tag="w2h", name=f"w2h_{e}")
                w2l = w_p.tile([P, DT, KH // 2, 2, P], f8, tag="w2l", name=f"w2l_{e}")
                nc.sync.dma_start(w2h[:], w2h_d[e])
                nc.sync.dma_start(w2l[:], w2l_d[e])
                return (w2h, w2l)

            def load_xs(c):
                (ci, e, off, w, s1) = c
                xs = xs_p.tile([P, KD, 2, CW], f8, tag="xs", name=f"xs_{ci}")
                nc.sync.dma_start(xs[:, :, :, :w], xs_d[:, :, :, off : off + w])
                return xs

            # PE pstate warmup: a few dependency-free matmuls at t=0 start
            # the cost model's ramp clock so the real matmuls (first data
            # lands ~5us later) run at full rate immediately
            dz = b1_p.tile([P, 2, P], f8, name="warmz")
            nc.vector.memset(dz[:], 0)
            wp = ps1.tile([P, CW], f32, tag="ps1", name="warmp")
            for _ in range(3):
                nc.tensor.matmul(wp[:, :P], dz[:], dz[:], start=True,
                                 stop=True, perf_mode=DR)

            # prologue issue order: the bytes GEMM1(chunk 0, hm 0, k<4)
            # needs come first, in fine-grained pieces
            e0 = chunks[0][1]
            w0 = chunks[0][3]
            w1h0 = w_p.tile([P, KH, KD, P], f8, tag="w1h", name=f"w1h_{e0}")
            nc.sync.dma_start(w1h0[:, : KH // 2], w1h_d[e0, :, : KH // 2])
            xs0 = xs_p.tile([P, KD, 2, CW], f8, tag="xs", name="xs_0")
            nc.sync.dma_start(
                xs0[:, : KD // 2, :, :w0], xs_d[:, : KD // 2, :, :w0]
            )
            nc.sync.dma_start(
                xs0[:, KD // 2 :, :, :w0], xs_d[:, KD // 2 :, :, :w0]
            )
            w1l0 = w_p.tile([P, KH, KD // 2, 2, P], f8, tag="w1l", name=f"w1l_{e0}")
            nc.sync.dma_start(w1l0[:, : KH // 2], w1l_d[e0, :, : KH // 2])
            b1s = b1_p.tile([P, E * KH], f32, name="b1s")
            nc.sync.dma_start(b1s[:], b1_d[:])
            nc.sync.dma_start(w1h0[:, KH // 2 :], w1h_d[e0, :, KH // 2 :])
            nc.sync.dma_start(w1l0[:, KH // 2 :], w1l_d[e0, :, KH // 2 :])
            xmap = {0: xs0}
            wmap = {e0: (w1h0, w1l0)}
            if n > 1:
                xmap[1] = load_xs(chunks[1])
            wmap[e0] = wmap[e0] + load_w2(e0)
            for c in chunks[1:3]:
                if c[1] not in wmap:
                    wmap[c[1]] = load_w1(c[1]) + load_w2(c[1])
                if c[0] not in xmap:
                    xmap[c[0]] = load_xs(c)

            hsm = {}

            def emit_g1(c, tail=False):
                (ci, e, off, w, s1) = c
                hs = hs_p.tile([P, KH, 2, CW], f8, tag="hs", name=f"hs_{ci}")
                hsm[ci] = hs
                _g1(nc, ps1, c, xmap.pop(ci), wmap[e][0], wmap[e][1], b1s,
                    t_p, hs, tail=tail)

            def emit_g2(c, tail=False):
                (ci, e, off, w, s1) = c
                ob = ob_p.tile([P, DT, CW], bf16, tag="ob", name=f"ob_{ci}")
                ring = nc.sync if tail else nc.scalar  # tail: idle SP ring

                def st(lo, hi):
                    ring.dma_start(
                        out_d[:, lo:hi, off : off + w], ob[:, lo:hi, :w]
                    )

                _g2(nc, ps2, c, hsm.pop(ci), wmap[e][2], wmap[e][3], ob,
                    store=(st, DT // 2 if tail else DT))

            # software pipeline: G1(i+1) is emitted before G2(i) so the PE
            # never waits on the ACT/Pool/DVE hid-split chain; at the tail
            # the last G1 is emitted two steps early (depth-2) since the
            # final chunks are narrow.
            emit_g1(chunks[0])
            g1p = 1
            for i in range(n):
                if i + 2 < n:
                    c2 = chunks[i + 2]
                    xmap[c2[0]] = load_xs(c2)
                    if c2[1] not in wmap:
                        wmap[c2[1]] = load_w1(c2[1]) + load_w2(c2[1])
                tgt = min(n - 1, i + 1 if i != n - 3 else n - 1)
                while g1p <= tgt:
                    emit_g1(chunks[g1p], tail=(g1p >= n - 2))
                    g1p += 1
                emit_g2(chunks[i], tail=(i >= n - 3))
                ce = chunks[i][1]
                if i + 1 == n or chunks[i + 1][1] != ce:
                    del wmap[ce]

    nc.compile()
    _compiled[key] = nc
    return nc


def _quant_split(a):
    """e4m3 hi/lo split of a pre-scaled float32 array."""
    hi = a.astype(F8)
    lo = (a - hi.astype(np.float32)).astype(F8)
    return hi, lo


def _pow2_scale(maxval, target=160.0):
    return float(2.0 ** np.floor(np.log2(target / maxval)))


def kernel(x, Wg, bg, W1, b1, W2, b2):
    global LAST_RESULT
    x = np.ascontiguousarray(x, dtype=np.float32)
    B, S, d = x.shape
    assert d == D
    T = B * S
    xf = x.reshape(T, d)

    # ---- Host gating/routing (fp64) ----
    logits = xf.astype(np.float64) @ np.asarray(Wg, np.float64) + np.asarray(
        bg, np.float64
    )
    mx = logits.max(axis=1, keepdims=True)
    ex = np.exp(logits - mx)
    probs = ex / ex.sum(axis=1, keepdims=True)
    order = np.argsort(-logits, axis=1, kind="stable")  # ties -> lower index
    top = order[:, :TOPK]  # [T, 2]
    gsel = np.take_along_axis(probs, top, axis=1).astype(np.float32)

    toks, gates = [], []
    for e in range(E):
        pos = top == e  # [T, 2]
        sel = pos.any(axis=1)
        toks.append(np.nonzero(sel)[0])
        gates.append((gsel * pos).sum(axis=1)[sel].astype(np.float32))

    # pair layout: expert-major (chunk widths are unconstrained; all AP
    # steps are fixed by the tile layouts)
    cnt = [len(t) for t in toks]
    cnt16 = list(cnt)
    offs = np.concatenate([[0], np.cumsum(cnt16)]).astype(np.int64)
    npp = int(offs[-1])
    pair_tok = np.zeros(npp, np.int64)
    for e in range(E):
        pair_tok[offs[e] : offs[e] + cnt[e]] = toks[e]

    # ---- scales (powers of 2; lossless to apply) ----
    sx = _pow2_scale(np.abs(xf).max())
    sw1 = _pow2_scale(np.abs(W1).max())
    sw2 = _pow2_scale(np.abs(W2).max())
    s1 = SH / (sx * sw1)  # ACT scale: psum1 -> hid*SH
    inv_out = 1.0 / (SH * sw2)

    # chunk widths: prefer full 512s (512B DMA descriptors); keep every
    # chunk >= 256 so the next chunk's GEMM1 always covers the hid-split
    # chain latency (split a trailing 512+r when the remainder is small)
    def plan_widths(tot):
        n512, r = divmod(tot, CW)
        if r == 0:
            ws = [CW] * n512
        elif r >= 256 or n512 == 0:
            ws = [CW] * n512 + [r]
        else:
            half = (CW + r) // 2 // 16 * 16
            ws = [CW] * (n512 - 1) + [half, CW + r - half]
        return ws

    widths = []
    for e in range(E):
        widths.append(plan_widths(cnt16[e]))
    # split the very last chunk so the final GEMM2 is covered by a GEMM1
    lw = widths[-1][-1]
    if lw >= 256:
        widths[-1] = widths[-1][:-1] + [lw - 128, 128]

    chunks = []
    ci = 0
    for e in range(E):
        off = int(offs[e])
        for w in widths[e]:
            chunks.append((ci, e, off, w, s1))
            ci += 1
            off += w

    # ---- x: gather pairs, scale, split, arrange [P, KD, 2, npp] ----
    xg = xf[pair_tok] * sx
    xh, xl = _quant_split(xg)
    xs_host = np.empty((P, KD, 2, npp), F8)
    xs_host[:, :, 0, :] = xh.reshape(npp, KD, P).transpose(2, 1, 0)
    xs_host[:, :, 1, :] = xl.reshape(npp, KD, P).transpose(2, 1, 0)

    # ---- per-core weight shards ----
    W1f = np.asarray(W1, np.float32) * sw1
    W2f = np.asarray(W2, np.float32) * sw2
    b1f = np.asarray(b1, np.float32) * SH
    core_maps = []
    for c in range(NCORES):
        sl = slice(c * HS, (c + 1) * HS)
        w1hi, w1lo = _quant_split(W1f[:, :, sl])  # [E, D, HS]
        w2hi, w2lo = _quant_split(W2f[:, sl, :])  # [E, HS, D]
        # GEMM1 stationary: [e, p(d-in-k), hm, k, j(h-in-hm)] (hi, no dup —
        # the device broadcasts the DoubleRow slot pair with a stride-0 AP)
        a = w1hi.reshape(E, KD, P, KH, P).transpose(0, 2, 3, 1, 4)  # [E,p,hm,k,j]
        w1h_host = np.ascontiguousarray(a)
        bl = w1lo.reshape(E, KD, P, KH, P).transpose(0, 2, 3, 1, 4)
        w1l_host = np.ascontiguousarray(bl.reshape(E, P, KH, KD // 2, 2, P))
        # GEMM2 stationary: [e, p(h-in-k), dt, k, j(d-in-dt)]
        a2 = w2hi.reshape(E, KH, P, DT, P).transpose(0, 2, 3, 1, 4)  # [E,p,dt,k,j]
        w2h_host = np.ascontiguousarray(a2)
        b2l = w2lo.reshape(E, KH, P, DT, P).transpose(0, 2, 3, 1, 4)
        w2l_host = np.ascontiguousarray(b2l.reshape(E, P, DT, KH // 2, 2, P))
        b1_host = np.ascontiguousarray(
            b1f[:, sl].reshape(E, KH, P).transpose(2, 0, 1).reshape(P, E * KH)
        )
        core_maps.append(
            {
                "xs": xs_host,
                "w1h": w1h_host,
                "w1l": w1l_host,
                "w2h": w2h_host,
                "w2l": w2l_host,
                "b1": b1_host,
            }
        )

    nc = _build(chunks, npp)
    res = run_bass_kernel_spmd(nc, core_maps, core_ids=list(range(NCORES)))
    LAST_RESULT = res

    # ---- combine partials on host ----
    total = np.zeros((P, DT, npp), np.float32)
    for c in range(NCORES):
        total += np.asarray(res.results[c]["out"]).astype(np.float32)
    # [p, dt, pair] -> [pair, dt*128=d]
    ytot = total.transpose(2, 1, 0).reshape(npp, D) * inv_out

    out = np.zeros((T, D), np.float32)
    b2f = np.asarray(b2, np.float32)
    for e in range(E):
        if cnt[e] == 0:
            continue
        ye = ytot[offs[e] : offs[e] + cnt[e]]
        out[toks[e]] += gates[e][:, None] * (ye + b2f[e])
    return out.reshape(B, S, D)
